# revision 19
# baseline (speedup 1.0000x reference)
"""Trainium2 Bass kernel for single-head attention (MDTA-style block).

Reference computation (per batch b, N=4096 tokens, C=128 channels):
    qkv = x @ W_fc + b_fc ; q,k,v = split(qkv)
    S   = (q @ k^T) / sqrt(C)
    A   = softmax(S / scale, axis=-1)
    out = (A @ v) @ W_out + b_out

Sharding: 8 cores = 4 batches x 2 query-halves (data parallel, no
cross-core comm). Each core computes 2048 query rows against the full
4096 keys/values of its batch.

Per-core algorithm (flash-style, NxN never hits HBM), v2:
  - All projections in bf16 (fp32 matmul is 4 cyc/row on PE; bf16 is 1).
  - k-bias and the additive per-query terms cancel in softmax, so
    kT = Wk^T x^T with NO bias; only q keeps its bias.
  - W_out is folded into the v-projection: P = x @ (Wv @ W_out), so the
    attention output-projection stage disappears. The output is
    accumulated directly as y^T = sum_k E[k,:] P[k,:] in PSUM.
  - scores computed TRANSPOSED per 128-key tile, 512-query block, into
    3-key-tile PSUM groups [128, 3*512] so one ScalarE activation
    covers 1536 elements/lane (amortizes the ~185ns/inst overhead).
  - exp on ScalarE emits E directly in fp8e4m3 with a fused scale and a
    constant -2 shift (softmax-invariant) to stay under fp8 max (240).
  - A@V and the row-sum (ones) matmuls run in fp8 DoubleRow mode
    (K=256 per matmul, 0.5 cyc/row): E pairs [128, 2, 512] against
    V pairs / ones [128, 2, 128].
  - normalize with VectorE reciprocal + multiply + per-partition bias
    b2 = bv @ W_out + b_out; y stored transposed [C, NQ], host flips.
"""

import math
import sys

import numpy as np

sys.path.insert(0, "/opt/trn_rl_repo")

import ml_dtypes  # noqa: E402

import concourse.bacc as bacc  # noqa: E402
import concourse.mybir as mybir  # noqa: E402
import concourse.tile as tile  # noqa: E402
from concourse.bass_utils import run_bass_kernel_spmd  # noqa: E402

B, N, C = 4, 4096, 128
NCORES = 8
NQ = N // 2  # queries per core
NB = 512  # query block size
NBLK = NQ // NB  # 4
NMT = N // C  # key tiles (32)
NPAIR = NMT // 2  # DoubleRow key-tile pairs (16)
GSZ = 3  # key tiles per activation group
SHIFT = -4.0  # exp(x - 4): softmax-invariant, keeps E < fp8e4m3 max (240)
F32 = mybir.dt.float32
BF16 = mybir.dt.bfloat16
F8 = mybir.dt.float8e4
DR = mybir.MatmulPerfMode.DoubleRow

_cache: dict = {}
LAST_RESULTS = None


def _build(sc: float):
    nc = bacc.Bacc(None, target_bir_lowering=False, debug=True)

    xT = nc.declare_dram_parameter("xT", [C, N], BF16, isOutput=False)
    xTq = nc.declare_dram_parameter("xTq", [C, NQ], BF16, isOutput=False)
    Wq = nc.declare_dram_parameter("Wq", [C, C], BF16, isOutput=False)
    Wk = nc.declare_dram_parameter("Wk", [C, C], BF16, isOutput=False)
    WP = nc.declare_dram_parameter("WP", [C, C], BF16, isOutput=False)
    bq = nc.declare_dram_parameter("bq", [C, 1], F32, isOutput=False)
    b2 = nc.declare_dram_parameter("b2", [C, 1], F32, isOutput=False)
    sh = nc.declare_dram_parameter("sh", [C, 1], F32, isOutput=False)
    ones = nc.declare_dram_parameter("ones", [C, 2, C], F8, isOutput=False)
    y = nc.declare_dram_parameter("y", [C, NQ], F32, isOutput=True)

    with tile.TileContext(nc) as tc:
        with (
            tc.tile_pool(name="const", bufs=1) as cp,
            tc.tile_pool(name="ebuf", bufs=2) as ep,
            tc.tile_pool(name="nrm", bufs=2) as sp,
            tc.tile_pool(name="ps", bufs=2, space="PSUM") as psp,
            tc.tile_pool(name="ps_o", bufs=1, space="PSUM") as pop,
            tc.tile_pool(name="ps_d", bufs=1, space="PSUM") as pdp,
        ):
            xT_s = cp.tile([C, N], BF16)
            xTq_s = cp.tile([C, NQ], BF16)
            wq_s = cp.tile([C, C], BF16)
            wk_s = cp.tile([C, C], BF16)
            wp_s = cp.tile([C, C], BF16)
            bq_s = cp.tile([C, 1], F32)
            b2_s = cp.tile([C, 1], F32)
            sh_s = cp.tile([C, 1], F32)
            ones_s = cp.tile([C, 2, C], F8)
            kT_s = cp.tile([C, N], BF16)
            tT_s = cp.tile([C, NQ], BF16)
            # two-level fp8 V: P ~= V_h + V_l (residual), bf16-class accuracy
            # at fp8-DoubleRow matmul speed
            V_h = cp.tile([C, NMT, C], F8)
            V_l = cp.tile([C, NMT, C], F8)

            # Parallel DMA prologue: q-half x and small params go out on the
            # GpSimd (SWDGE) queue, k/v-side x on the SP queue, ordered so
            # the first scores group can start a few us in. (Each dma_start
            # costs ~0.6us serially on its issuing sequencer.)
            for dst, src in [
                (wq_s, Wq), (sh_s, sh), (bq_s, bq),
            ]:
                nc.gpsimd.dma_start(out=dst[:], in_=src[:])
            XCH = 1024
            for ch in range(NQ // XCH):
                sl = slice(ch * XCH, (ch + 1) * XCH)
                nc.gpsimd.dma_start(out=xTq_s[:, sl], in_=xTq[:, sl])
            for dst, src in [(wp_s, WP), (b2_s, b2), (ones_s, ones)]:
                nc.gpsimd.dma_start(out=dst[:], in_=src[:])
            nc.sync.dma_start(out=wk_s[:], in_=Wk[:])
            for ch in range(N // XCH):
                sl = slice(ch * XCH, (ch + 1) * XCH)
                nc.sync.dma_start(out=xT_s[:, sl], in_=xT[:, sl])

            def kt_chunk(ch):
                sl = slice(ch * NB, (ch + 1) * NB)
                ps = psp.tile([C, NB], F32, tag="ps")
                nc.tensor.matmul(ps[:], wk_s[:], xT_s[:, sl], start=True, stop=True)
                nc.vector.tensor_copy(kT_s[:, sl], ps[:])

            def tt_chunk(ch):
                sl = slice(ch * NB, (ch + 1) * NB)
                ps = psp.tile([C, NB], F32, tag="ps")
                nc.tensor.matmul(ps[:], wq_s[:], xTq_s[:, sl], start=True, stop=True)
                nc.vector.tensor_scalar_add(tT_s[:, sl], ps[:], bq_s[:])

            # kT = Wk^T @ x^T (no bias: its softmax contribution cancels);
            # tT = Wq^T @ xq^T + bq. Emitted just-in-time between block-0
            # score groups (see below) so exp starts as early as possible.

            def v_tile(mt):
                # P = x @ (Wv @ W_out) in two-level fp8; psum ping-pongs
                # through the pso/psd banks so the "ps" score pool isn't
                # serialized behind the V tiles.
                msl = slice(mt * C, (mt + 1) * C)
                vpool, vtag = (pop, "pso") if mt % 2 == 0 else (pdp, "psd")
                psv = vpool.tile([C, C], F32, tag=vtag)
                nc.tensor.matmul(psv[:], xT_s[:, msl], wp_s[:], start=True, stop=True)
                nc.vector.tensor_copy(V_h[:, mt, :], psv[:])
                nc.vector.tensor_tensor(
                    V_l[:, mt, :], psv[:], V_h[:, mt, :],
                    op=mybir.AluOpType.subtract,
                )

            groups = [(g * GSZ, min(GSZ, NMT - g * GSZ))
                      for g in range((NMT + GSZ - 1) // GSZ)]

            def scores_exp(nb, E, pre_group=None):
                qsl = slice(nb * NB, (nb + 1) * NB)
                for gi, (t0, gsz) in enumerate(groups):
                    if pre_group is not None:
                        pre_group(gi)
                    psg = psp.tile([C, GSZ, NB], F32, tag="ps")
                    for j in range(gsz):
                        nc.tensor.matmul(
                            psg[:, j, :],
                            kT_s[:, (t0 + j) * C:(t0 + j + 1) * C],
                            tT_s[:, qsl],
                            start=True, stop=True,
                        )
                    nc.scalar.activation(
                        E[:, t0:t0 + gsz, :], psg[:, :gsz, :],
                        mybir.ActivationFunctionType.Exp,
                        bias=sh_s[:], scale=sc,
                    )

            def av_norm(nb, E):
                qsl = slice(nb * NB, (nb + 1) * NB)
                pso = pop.tile([C, NB], F32, tag="pso")
                psd = pdp.tile([C, NB], F32, tag="psd")
                for t in range(NPAIR):
                    e2 = E[:, 2 * t:2 * t + 2, :]
                    nc.tensor.matmul(
                        pso[:], V_h[:, 2 * t:2 * t + 2, :], e2,
                        start=(t == 0), stop=False, perf_mode=DR,
                    )
                    nc.tensor.matmul(
                        pso[:], V_l[:, 2 * t:2 * t + 2, :], e2,
                        start=False, stop=(t == NPAIR - 1), perf_mode=DR,
                    )
                    nc.tensor.matmul(
                        psd[:], ones_s[:], e2,
                        start=(t == 0), stop=(t == NPAIR - 1), perf_mode=DR,
                    )
                rcp = sp.tile([C, NB], F32, tag="rcp")
                nc.vector.reciprocal_approx_fast(rcp[:], psd[:])
                yt = sp.tile([C, NB], F32, tag="yt")
                nc.vector.tensor_tensor(yt[:], pso[:], rcp[:], op=mybir.AluOpType.mult)
                yb = sp.tile([C, NB], F32, tag="yb")
                nc.vector.tensor_scalar_add(yb[:], yt[:], b2_s[:])
                nc.sync.dma_start(out=y[:, qsl], in_=yb[:])

            # Software pipeline: emit AV/normalize of block b after the
            # scores+exp of block b+1 so the PE always has score matmuls
            # ready and ScalarE (the bottleneck) never starves.
            Es = [ep.tile([C, NMT, NB], F8, tag="E", name=f"E{i}")
                  for i in range(2)]
            E_of = lambda nb: Es[nb % 2]

            # Block 0: weave the kT/tT projection chunks between the score
            # groups just-in-time (group g needs kT tiles 3g..3g+2, i.e.
            # kT chunks up to (3g+2)//4; all of block 0 needs only tT c0).
            kt_emitted = [0]

            def pre_group0(gi):
                if gi == 0:
                    kt_chunk(0)
                    tt_chunk(0)
                    kt_emitted[0] = 1
                need = min((3 * gi + 2) // 4 + 1, N // NB)
                while kt_emitted[0] < need:
                    kt_chunk(kt_emitted[0])
                    kt_emitted[0] += 1

            scores_exp(0, E_of(0), pre_group=pre_group0)
            while kt_emitted[0] < N // NB:
                kt_chunk(kt_emitted[0])
                kt_emitted[0] += 1
            for ch in range(1, NQ // NB):
                tt_chunk(ch)
            for mt in range(NMT):
                v_tile(mt)
            for nb in range(1, NBLK):
                scores_exp(nb, E_of(nb))
                av_norm(nb - 1, E_of(nb - 1))
            av_norm(NBLK - 1, E_of(NBLK - 1))

    nc.compile()
    return nc


def kernel(x, W_fc, b_fc, W_out, b_out, scale):
    x = np.asarray(x, dtype=np.float32)
    W_fc = np.asarray(W_fc, dtype=np.float32)
    b_fc = np.asarray(b_fc, dtype=np.float32)
    W_out = np.asarray(W_out, dtype=np.float32)
    b_out = np.asarray(b_out, dtype=np.float32)
    scale = np.asarray(scale, dtype=np.float32)

    sc = float(1.0 / (math.sqrt(C) * float(scale[0])))
    key = ("v2", sc)
    if key not in _cache:
        _cache.clear()
        _cache[key] = _build(sc)
    nc = _cache[key]

    b2 = b_fc[2 * C:] @ W_out + b_out  # v-bias folded through the projection
    common = {
        "Wq": np.ascontiguousarray(W_fc[:, :C]).astype(ml_dtypes.bfloat16),
        "Wk": np.ascontiguousarray(W_fc[:, C:2 * C]).astype(ml_dtypes.bfloat16),
        "WP": (W_fc[:, 2 * C:] @ W_out).astype(ml_dtypes.bfloat16),
        "bq": np.ascontiguousarray(b_fc[:C].reshape(C, 1)),
        "b2": np.ascontiguousarray(b2.reshape(C, 1)),
        "sh": np.full((C, 1), SHIFT, dtype=np.float32),
        "ones": np.ones((C, 2, C), dtype=ml_dtypes.float8_e4m3),
    }
    in_maps = []
    for core in range(NCORES):
        b, h = core // 2, core % 2
        xT_b = np.ascontiguousarray(x[b].T.astype(ml_dtypes.bfloat16))
        xTq_b = np.ascontiguousarray(
            x[b, h * NQ:(h + 1) * NQ, :].T.astype(ml_dtypes.bfloat16))
        in_maps.append({**common, "xT": xT_b, "xTq": xTq_b})

    res = run_bass_kernel_spmd(nc, in_maps, list(range(NCORES)))
    global LAST_RESULTS
    LAST_RESULTS = res

    y = np.empty((B, N, C), dtype=np.float32)
    for core in range(NCORES):
        b, h = core // 2, core % 2
        y[b, h * NQ:(h + 1) * NQ, :] = res.results[core]["y"].T
    return y


# revision 22
# speedup vs baseline: 1.2492x; 1.2492x over previous
"""Trainium2 Bass kernel for single-head attention (MDTA-style block).

Reference computation (per batch b, N=4096 tokens, C=128 channels):
    qkv = x @ W_fc + b_fc ; q,k,v = split(qkv)
    S   = (q @ k^T) / sqrt(C)
    A   = softmax(S / scale, axis=-1)
    out = (A @ v) @ W_out + b_out

Sharding: 8 cores = 4 batches x 2 query-halves (data parallel, no
cross-core comm). Each core computes 2048 query rows against the full
4096 keys/values of its batch.

Per-core algorithm (flash-style, NxN never hits HBM), v2:
  - All projections in bf16 (fp32 matmul is 4 cyc/row on PE; bf16 is 1).
  - k-bias and the additive per-query terms cancel in softmax, so
    kT = Wk^T x^T with NO bias; only q keeps its bias.
  - W_out is folded into the v-projection: P = x @ (Wv @ W_out), so the
    attention output-projection stage disappears. The output is
    accumulated directly as y^T = sum_k E[k,:] P[k,:] in PSUM.
  - scores computed TRANSPOSED per 128-key tile, 512-query block, into
    3-key-tile PSUM groups [128, 3*512] so one ScalarE activation
    covers 1536 elements/lane (amortizes the ~185ns/inst overhead).
  - exp on ScalarE emits E directly in fp8e4m3 with a fused scale and a
    constant -2 shift (softmax-invariant) to stay under fp8 max (240).
  - A@V and the row-sum (ones) matmuls run in fp8 DoubleRow mode
    (K=256 per matmul, 0.5 cyc/row): E pairs [128, 2, 512] against
    V pairs / ones [128, 2, 128].
  - normalize with VectorE reciprocal + multiply + per-partition bias
    b2 = bv @ W_out + b_out; y stored transposed [C, NQ], host flips.
"""

import math
import sys

import numpy as np

sys.path.insert(0, "/opt/trn_rl_repo")

import ml_dtypes  # noqa: E402

import concourse.bacc as bacc  # noqa: E402
import concourse.mybir as mybir  # noqa: E402
import concourse.tile as tile  # noqa: E402
from concourse.bass_utils import run_bass_kernel_spmd  # noqa: E402

B, N, C = 4, 4096, 128
NCORES = 8
NQ = N // 2  # queries per core
NB = 512  # query block size
NBLK = NQ // NB  # 4
NMT = N // C  # key tiles (32)
NPAIR = NMT // 2  # DoubleRow key-tile pairs (16)
GSZ = 2  # key tiles per activation group
SHIFT = -4.0  # exp(x - 4): softmax-invariant, keeps E < fp8e4m3 max (240)
F32 = mybir.dt.float32
BF16 = mybir.dt.bfloat16
F8 = mybir.dt.float8e4
DR = mybir.MatmulPerfMode.DoubleRow

_cache: dict = {}
LAST_RESULTS = None


def _build(sc: float):
    nc = bacc.Bacc(None, target_bir_lowering=False, debug=True)

    xT = nc.declare_dram_parameter("xT", [C, N], BF16, isOutput=False)
    xTq = nc.declare_dram_parameter("xTq", [C, NQ], BF16, isOutput=False)
    Wq = nc.declare_dram_parameter("Wq", [C, C], BF16, isOutput=False)
    Wk = nc.declare_dram_parameter("Wk", [C, C], BF16, isOutput=False)
    WP = nc.declare_dram_parameter("WP", [C, C], BF16, isOutput=False)
    bq = nc.declare_dram_parameter("bq", [C, 1], F32, isOutput=False)
    b2 = nc.declare_dram_parameter("b2", [C, 1], F32, isOutput=False)
    sh = nc.declare_dram_parameter("sh", [C, 1], F32, isOutput=False)
    ones = nc.declare_dram_parameter("ones", [C, 2, C], F8, isOutput=False)
    y = nc.declare_dram_parameter("y", [C, NQ], F32, isOutput=True)

    with tile.TileContext(nc) as tc:
        with (
            tc.tile_pool(name="const", bufs=1) as cp,
            tc.tile_pool(name="ebuf", bufs=2) as ep,
            tc.tile_pool(name="nrm", bufs=2) as sp,
            tc.tile_pool(name="ps", bufs=3, space="PSUM") as psp,
            tc.tile_pool(name="ps_o", bufs=1, space="PSUM") as pop,
            tc.tile_pool(name="ps_d", bufs=1, space="PSUM") as pdp,
        ):
            xT_s = cp.tile([C, N], BF16)
            xTq_s = cp.tile([C, NQ], BF16)
            wq_s = cp.tile([C, C], BF16)
            wk_s = cp.tile([C, C], BF16)
            wp_s = cp.tile([C, C], BF16)
            bq_s = cp.tile([C, 1], F32)
            b2_s = cp.tile([C, 1], F32)
            sh_s = cp.tile([C, 1], F32)
            ones_s = cp.tile([C, 2, C], F8)
            kT_s = cp.tile([C, N], BF16)
            tT_s = cp.tile([C, NQ], BF16)
            # two-level fp8 V: P ~= V_h + V_l (residual), bf16-class accuracy
            # at fp8-DoubleRow matmul speed
            V_h = cp.tile([C, NMT, C], F8)
            V_l = cp.tile([C, NMT, C], F8)

            # Parallel DMA prologue: q-half x and small params go out on the
            # GpSimd (SWDGE) queue, k/v-side x on the SP queue, ordered so
            # the first scores group can start a few us in. (Each dma_start
            # costs ~0.6us serially on its issuing sequencer.)
            for dst, src in [
                (wq_s, Wq), (sh_s, sh), (bq_s, bq),
            ]:
                nc.gpsimd.dma_start(out=dst[:], in_=src[:])
            XCH = 1024
            for ch in range(NQ // XCH):
                sl = slice(ch * XCH, (ch + 1) * XCH)
                nc.gpsimd.dma_start(out=xTq_s[:, sl], in_=xTq[:, sl])
            for dst, src in [(wp_s, WP), (b2_s, b2), (ones_s, ones)]:
                nc.gpsimd.dma_start(out=dst[:], in_=src[:])
            nc.sync.dma_start(out=wk_s[:], in_=Wk[:])
            for ch in range(N // XCH):
                sl = slice(ch * XCH, (ch + 1) * XCH)
                nc.sync.dma_start(out=xT_s[:, sl], in_=xT[:, sl])

            def kt_chunk(ch):
                sl = slice(ch * NB, (ch + 1) * NB)
                ps = psp.tile([C, NB], F32, tag="ps")
                nc.tensor.matmul(ps[:], wk_s[:], xT_s[:, sl], start=True, stop=True)
                nc.vector.tensor_copy(kT_s[:, sl], ps[:])

            def tt_chunk(ch):
                sl = slice(ch * NB, (ch + 1) * NB)
                ps = psp.tile([C, NB], F32, tag="ps")
                nc.tensor.matmul(ps[:], wq_s[:], xTq_s[:, sl], start=True, stop=True)
                nc.vector.tensor_scalar_add(tT_s[:, sl], ps[:], bq_s[:])

            # kT = Wk^T @ x^T (no bias: its softmax contribution cancels);
            # tT = Wq^T @ xq^T + bq. Emitted just-in-time between block-0
            # score groups (see below) so exp starts as early as possible.

            def v_tile(mt):
                # P = x @ (Wv @ W_out) in two-level fp8; psum ping-pongs
                # through the pso/psd banks so the "ps" score pool isn't
                # serialized behind the V tiles.
                msl = slice(mt * C, (mt + 1) * C)
                vpool, vtag = (pop, "pso") if mt % 2 == 0 else (pdp, "psd")
                psv = vpool.tile([C, C], F32, tag=vtag)
                nc.tensor.matmul(psv[:], xT_s[:, msl], wp_s[:], start=True, stop=True)
                nc.vector.tensor_copy(V_h[:, mt, :], psv[:])
                nc.vector.tensor_tensor(
                    V_l[:, mt, :], psv[:], V_h[:, mt, :],
                    op=mybir.AluOpType.subtract,
                )

            groups = [(g * GSZ, min(GSZ, NMT - g * GSZ))
                      for g in range((NMT + GSZ - 1) // GSZ)]

            def scores_exp(nb, E, pre_group=None):
                qsl = slice(nb * NB, (nb + 1) * NB)
                for gi, (t0, gsz) in enumerate(groups):
                    if pre_group is not None:
                        pre_group(gi)
                    psg = psp.tile([C, GSZ, NB], F32, tag="ps")
                    for j in range(gsz):
                        nc.tensor.matmul(
                            psg[:, j, :],
                            kT_s[:, (t0 + j) * C:(t0 + j + 1) * C],
                            tT_s[:, qsl],
                            start=True, stop=True,
                        )
                    nc.scalar.activation(
                        E[:, t0:t0 + gsz, :], psg[:, :gsz, :],
                        mybir.ActivationFunctionType.Exp,
                        bias=sh_s[:], scale=sc,
                    )

            def av_norm(nb, E):
                qsl = slice(nb * NB, (nb + 1) * NB)
                pso = pop.tile([C, NB], F32, tag="pso")
                psd = pdp.tile([C, NB], F32, tag="psd")
                for t in range(NPAIR):
                    e2 = E[:, 2 * t:2 * t + 2, :]
                    nc.tensor.matmul(
                        pso[:], V_h[:, 2 * t:2 * t + 2, :], e2,
                        start=(t == 0), stop=False, perf_mode=DR,
                    )
                    nc.tensor.matmul(
                        pso[:], V_l[:, 2 * t:2 * t + 2, :], e2,
                        start=False, stop=(t == NPAIR - 1), perf_mode=DR,
                    )
                    nc.tensor.matmul(
                        psd[:], ones_s[:], e2,
                        start=(t == 0), stop=(t == NPAIR - 1), perf_mode=DR,
                    )
                rcp = sp.tile([C, NB], F32, tag="rcp")
                nc.vector.reciprocal_approx_fast(rcp[:], psd[:])
                yt = sp.tile([C, NB], F32, tag="yt")
                nc.vector.tensor_tensor(yt[:], pso[:], rcp[:], op=mybir.AluOpType.mult)
                yb = sp.tile([C, NB], F32, tag="yb")
                nc.vector.tensor_scalar_add(yb[:], yt[:], b2_s[:])
                nc.sync.dma_start(out=y[:, qsl], in_=yb[:])

            # Software pipeline: emit AV/normalize of block b after the
            # scores+exp of block b+1 so the PE always has score matmuls
            # ready and ScalarE (the bottleneck) never starves.
            Es = [ep.tile([C, NMT, NB], F8, tag="E", name=f"E{i}")
                  for i in range(2)]
            E_of = lambda nb: Es[nb % 2]

            kt_chunk(0)
            tt_chunk(0)
            for ch in range(1, N // NB):
                kt_chunk(ch)
            for ch in range(1, NQ // NB):
                tt_chunk(ch)
            scores_exp(0, E_of(0))
            for mt in range(NMT):
                v_tile(mt)
            for nb in range(1, NBLK):
                scores_exp(nb, E_of(nb))
                av_norm(nb - 1, E_of(nb - 1))
            av_norm(NBLK - 1, E_of(NBLK - 1))

    nc.compile()
    return nc


def kernel(x, W_fc, b_fc, W_out, b_out, scale):
    x = np.asarray(x, dtype=np.float32)
    W_fc = np.asarray(W_fc, dtype=np.float32)
    b_fc = np.asarray(b_fc, dtype=np.float32)
    W_out = np.asarray(W_out, dtype=np.float32)
    b_out = np.asarray(b_out, dtype=np.float32)
    scale = np.asarray(scale, dtype=np.float32)

    sc = float(1.0 / (math.sqrt(C) * float(scale[0])))
    key = ("v2", sc)
    if key not in _cache:
        _cache.clear()
        _cache[key] = _build(sc)
    nc = _cache[key]

    b2 = b_fc[2 * C:] @ W_out + b_out  # v-bias folded through the projection
    common = {
        "Wq": np.ascontiguousarray(W_fc[:, :C]).astype(ml_dtypes.bfloat16),
        "Wk": np.ascontiguousarray(W_fc[:, C:2 * C]).astype(ml_dtypes.bfloat16),
        "WP": (W_fc[:, 2 * C:] @ W_out).astype(ml_dtypes.bfloat16),
        "bq": np.ascontiguousarray(b_fc[:C].reshape(C, 1)),
        "b2": np.ascontiguousarray(b2.reshape(C, 1)),
        "sh": np.full((C, 1), SHIFT, dtype=np.float32),
        "ones": np.ones((C, 2, C), dtype=ml_dtypes.float8_e4m3),
    }
    in_maps = []
    for core in range(NCORES):
        b, h = core // 2, core % 2
        xT_b = np.ascontiguousarray(x[b].T.astype(ml_dtypes.bfloat16))
        xTq_b = np.ascontiguousarray(
            x[b, h * NQ:(h + 1) * NQ, :].T.astype(ml_dtypes.bfloat16))
        in_maps.append({**common, "xT": xT_b, "xTq": xTq_b})

    res = run_bass_kernel_spmd(nc, in_maps, list(range(NCORES)))
    global LAST_RESULTS
    LAST_RESULTS = res

    y = np.empty((B, N, C), dtype=np.float32)
    for core in range(NCORES):
        b, h = core // 2, core % 2
        y[b, h * NQ:(h + 1) * NQ, :] = res.results[core]["y"].T
    return y


# revision 24
# speedup vs baseline: 1.2742x; 1.0200x over previous
"""Trainium2 Bass kernel for single-head attention (MDTA-style block).

Reference computation (per batch b, N=4096 tokens, C=128 channels):
    qkv = x @ W_fc + b_fc ; q,k,v = split(qkv)
    S   = (q @ k^T) / sqrt(C)
    A   = softmax(S / scale, axis=-1)
    out = (A @ v) @ W_out + b_out

Sharding: 8 cores = 4 batches x 2 query-halves (data parallel, no
cross-core comm). Each core computes 2048 query rows against the full
4096 keys/values of its batch.

Per-core algorithm (flash-style, NxN never hits HBM), v2:
  - All projections in bf16 (fp32 matmul is 4 cyc/row on PE; bf16 is 1).
  - k-bias and the additive per-query terms cancel in softmax, so
    kT = Wk^T x^T with NO bias; only q keeps its bias.
  - W_out is folded into the v-projection: P = x @ (Wv @ W_out), so the
    attention output-projection stage disappears. The output is
    accumulated directly as y^T = sum_k E[k,:] P[k,:] in PSUM.
  - scores computed TRANSPOSED per 128-key tile, 512-query block, into
    3-key-tile PSUM groups [128, 3*512] so one ScalarE activation
    covers 1536 elements/lane (amortizes the ~185ns/inst overhead).
  - exp on ScalarE emits E directly in fp8e4m3 with a fused scale and a
    constant -2 shift (softmax-invariant) to stay under fp8 max (240).
  - A@V and the row-sum (ones) matmuls run in fp8 DoubleRow mode
    (K=256 per matmul, 0.5 cyc/row): E pairs [128, 2, 512] against
    V pairs / ones [128, 2, 128].
  - normalize with VectorE reciprocal + multiply + per-partition bias
    b2 = bv @ W_out + b_out; y stored transposed [C, NQ], host flips.
"""

import math
import sys

import numpy as np

sys.path.insert(0, "/opt/trn_rl_repo")

import ml_dtypes  # noqa: E402

import concourse.bacc as bacc  # noqa: E402
import concourse.mybir as mybir  # noqa: E402
import concourse.tile as tile  # noqa: E402
from concourse.bass_utils import run_bass_kernel_spmd  # noqa: E402

B, N, C = 4, 4096, 128
NCORES = 8
NQ = N // 2  # queries per core
NB = 512  # query block size
NBLK = NQ // NB  # 4
NMT = N // C  # key tiles (32)
NPAIR = NMT // 2  # DoubleRow key-tile pairs (16)
GSZ = 2  # key tiles per activation group
SHIFT = -4.0  # exp(x - 4): softmax-invariant, keeps E < fp8e4m3 max (240)
F32 = mybir.dt.float32
BF16 = mybir.dt.bfloat16
F8 = mybir.dt.float8e4
DR = mybir.MatmulPerfMode.DoubleRow

_cache: dict = {}
LAST_RESULTS = None


def _build(sc: float):
    nc = bacc.Bacc(None, target_bir_lowering=False, debug=True)

    xT = nc.declare_dram_parameter("xT", [C, N], BF16, isOutput=False)
    xTq = nc.declare_dram_parameter("xTq", [C, NQ], BF16, isOutput=False)
    Wq = nc.declare_dram_parameter("Wq", [C, C], BF16, isOutput=False)
    Wk = nc.declare_dram_parameter("Wk", [C, C], BF16, isOutput=False)
    WP = nc.declare_dram_parameter("WP", [C, C], BF16, isOutput=False)
    bq = nc.declare_dram_parameter("bq", [C, 1], F32, isOutput=False)
    b2 = nc.declare_dram_parameter("b2", [C, 1], F32, isOutput=False)
    sh = nc.declare_dram_parameter("sh", [C, 1], F32, isOutput=False)
    ones = nc.declare_dram_parameter("ones", [C, 2, C], F8, isOutput=False)
    y = nc.declare_dram_parameter("y", [C, NQ], F32, isOutput=True)

    with tile.TileContext(nc) as tc:
        with (
            tc.tile_pool(name="const", bufs=1) as cp,
            tc.tile_pool(name="ebuf", bufs=2) as ep,
            tc.tile_pool(name="nrm", bufs=2) as sp,
            tc.tile_pool(name="ps", bufs=3, space="PSUM") as psp,
            tc.tile_pool(name="ps_o", bufs=1, space="PSUM") as pop,
            tc.tile_pool(name="ps_d", bufs=1, space="PSUM") as pdp,
        ):
            xT_s = cp.tile([C, N], BF16)
            xTq_s = cp.tile([C, NQ], BF16)
            wq_s = cp.tile([C, C], BF16)
            wk_s = cp.tile([C, C], BF16)
            wp_s = cp.tile([C, C], BF16)
            bq_s = cp.tile([C, 1], F32)
            b2_s = cp.tile([C, 1], F32)
            sh_s = cp.tile([C, 1], F32)
            ones_s = cp.tile([C, 2, C], F8)
            kT_s = cp.tile([C, N], BF16)
            tT_s = cp.tile([C, NQ], BF16)
            # two-level fp8 V: P ~= V_h + V_l (residual), bf16-class accuracy
            # at fp8-DoubleRow matmul speed
            V_h = cp.tile([C, NMT, C], F8)
            V_l = cp.tile([C, NMT, C], F8)

            # Parallel DMA prologue across four engine queues (each
            # dma_start costs ~0.6-0.8us serially on its issuing
            # sequencer), so all of x lands ~3us in and the first scores
            # group starts right behind it.
            XCH = 1024
            nc.sync.dma_start(out=wk_s[:], in_=Wk[:])
            nc.sync.dma_start(out=xT_s[:, 0:XCH], in_=xT[:, 0:XCH])
            nc.sync.dma_start(out=xT_s[:, XCH:2 * XCH], in_=xT[:, XCH:2 * XCH])
            nc.scalar.dma_start(out=sh_s[:], in_=sh[:])
            nc.scalar.dma_start(out=xTq_s[:, 0:XCH], in_=xTq[:, 0:XCH])
            nc.scalar.dma_start(out=xTq_s[:, XCH:2 * XCH], in_=xTq[:, XCH:2 * XCH])
            nc.gpsimd.dma_start(out=wq_s[:], in_=Wq[:])
            nc.gpsimd.dma_start(out=bq_s[:], in_=bq[:])
            nc.gpsimd.dma_start(
                out=xT_s[:, 2 * XCH:3 * XCH], in_=xT[:, 2 * XCH:3 * XCH])
            nc.gpsimd.dma_start(
                out=xT_s[:, 3 * XCH:4 * XCH], in_=xT[:, 3 * XCH:4 * XCH])
            for dst, src in [(wp_s, WP), (b2_s, b2), (ones_s, ones)]:
                nc.gpsimd.dma_start(out=dst[:], in_=src[:])

            def kt_chunk(ch):
                sl = slice(ch * NB, (ch + 1) * NB)
                ps = psp.tile([C, NB], F32, tag="ps")
                nc.tensor.matmul(ps[:], wk_s[:], xT_s[:, sl], start=True, stop=True)
                nc.vector.tensor_copy(kT_s[:, sl], ps[:])

            def tt_chunk(ch):
                sl = slice(ch * NB, (ch + 1) * NB)
                ps = psp.tile([C, NB], F32, tag="ps")
                nc.tensor.matmul(ps[:], wq_s[:], xTq_s[:, sl], start=True, stop=True)
                nc.vector.tensor_scalar_add(tT_s[:, sl], ps[:], bq_s[:])

            # kT = Wk^T @ x^T (no bias: its softmax contribution cancels);
            # tT = Wq^T @ xq^T + bq. Emitted just-in-time between block-0
            # score groups (see below) so exp starts as early as possible.

            def v_tile(mt):
                # P = x @ (Wv @ W_out) in two-level fp8; psum ping-pongs
                # through the pso/psd banks so the "ps" score pool isn't
                # serialized behind the V tiles.
                msl = slice(mt * C, (mt + 1) * C)
                vpool, vtag = (pop, "pso") if mt % 2 == 0 else (pdp, "psd")
                psv = vpool.tile([C, C], F32, tag=vtag)
                nc.tensor.matmul(psv[:], xT_s[:, msl], wp_s[:], start=True, stop=True)
                nc.vector.tensor_copy(V_h[:, mt, :], psv[:])
                nc.vector.tensor_tensor(
                    V_l[:, mt, :], psv[:], V_h[:, mt, :],
                    op=mybir.AluOpType.subtract,
                )

            groups = [(g * GSZ, min(GSZ, NMT - g * GSZ))
                      for g in range((NMT + GSZ - 1) // GSZ)]

            def scores_exp(nb, E, pre_group=None):
                qsl = slice(nb * NB, (nb + 1) * NB)
                for gi, (t0, gsz) in enumerate(groups):
                    if pre_group is not None:
                        pre_group(gi)
                    psg = psp.tile([C, GSZ, NB], F32, tag="ps")
                    for j in range(gsz):
                        nc.tensor.matmul(
                            psg[:, j, :],
                            kT_s[:, (t0 + j) * C:(t0 + j + 1) * C],
                            tT_s[:, qsl],
                            start=True, stop=True,
                        )
                    nc.scalar.activation(
                        E[:, t0:t0 + gsz, :], psg[:, :gsz, :],
                        mybir.ActivationFunctionType.Exp,
                        bias=sh_s[:], scale=sc,
                    )

            def av_norm(nb, E):
                qsl = slice(nb * NB, (nb + 1) * NB)
                pso = pop.tile([C, NB], F32, tag="pso")
                psd = pdp.tile([C, NB], F32, tag="psd")
                for t in range(NPAIR):
                    e2 = E[:, 2 * t:2 * t + 2, :]
                    nc.tensor.matmul(
                        pso[:], V_h[:, 2 * t:2 * t + 2, :], e2,
                        start=(t == 0), stop=False, perf_mode=DR,
                    )
                    nc.tensor.matmul(
                        pso[:], V_l[:, 2 * t:2 * t + 2, :], e2,
                        start=False, stop=(t == NPAIR - 1), perf_mode=DR,
                    )
                    nc.tensor.matmul(
                        psd[:], ones_s[:], e2,
                        start=(t == 0), stop=(t == NPAIR - 1), perf_mode=DR,
                    )
                rcp = sp.tile([C, NB], F32, tag="rcp")
                nc.vector.reciprocal_approx_fast(rcp[:], psd[:])
                yt = sp.tile([C, NB], F32, tag="yt")
                nc.vector.tensor_tensor(yt[:], pso[:], rcp[:], op=mybir.AluOpType.mult)
                yb = sp.tile([C, NB], F32, tag="yb")
                nc.vector.tensor_scalar_add(yb[:], yt[:], b2_s[:])
                nc.sync.dma_start(out=y[:, qsl], in_=yb[:])

            # Software pipeline: emit AV/normalize of block b after the
            # scores+exp of block b+1 so the PE always has score matmuls
            # ready and ScalarE (the bottleneck) never starves.
            Es = [ep.tile([C, NMT, NB], F8, tag="E", name=f"E{i}")
                  for i in range(2)]
            E_of = lambda nb: Es[nb % 2]

            kt_chunk(0)
            tt_chunk(0)
            for ch in range(1, N // NB):
                kt_chunk(ch)
            for ch in range(1, NQ // NB):
                tt_chunk(ch)
            scores_exp(0, E_of(0))
            for mt in range(NMT):
                v_tile(mt)
            for nb in range(1, NBLK):
                scores_exp(nb, E_of(nb))
                av_norm(nb - 1, E_of(nb - 1))
            av_norm(NBLK - 1, E_of(NBLK - 1))

    nc.compile()
    return nc


def kernel(x, W_fc, b_fc, W_out, b_out, scale):
    x = np.asarray(x, dtype=np.float32)
    W_fc = np.asarray(W_fc, dtype=np.float32)
    b_fc = np.asarray(b_fc, dtype=np.float32)
    W_out = np.asarray(W_out, dtype=np.float32)
    b_out = np.asarray(b_out, dtype=np.float32)
    scale = np.asarray(scale, dtype=np.float32)

    sc = float(1.0 / (math.sqrt(C) * float(scale[0])))
    key = ("v2", sc)
    if key not in _cache:
        _cache.clear()
        _cache[key] = _build(sc)
    nc = _cache[key]

    b2 = b_fc[2 * C:] @ W_out + b_out  # v-bias folded through the projection
    common = {
        "Wq": np.ascontiguousarray(W_fc[:, :C]).astype(ml_dtypes.bfloat16),
        "Wk": np.ascontiguousarray(W_fc[:, C:2 * C]).astype(ml_dtypes.bfloat16),
        "WP": (W_fc[:, 2 * C:] @ W_out).astype(ml_dtypes.bfloat16),
        "bq": np.ascontiguousarray(b_fc[:C].reshape(C, 1)),
        "b2": np.ascontiguousarray(b2.reshape(C, 1)),
        "sh": np.full((C, 1), SHIFT, dtype=np.float32),
        "ones": np.ones((C, 2, C), dtype=ml_dtypes.float8_e4m3),
    }
    in_maps = []
    for core in range(NCORES):
        b, h = core // 2, core % 2
        xT_b = np.ascontiguousarray(x[b].T.astype(ml_dtypes.bfloat16))
        xTq_b = np.ascontiguousarray(
            x[b, h * NQ:(h + 1) * NQ, :].T.astype(ml_dtypes.bfloat16))
        in_maps.append({**common, "xT": xT_b, "xTq": xTq_b})

    res = run_bass_kernel_spmd(nc, in_maps, list(range(NCORES)))
    global LAST_RESULTS
    LAST_RESULTS = res

    y = np.empty((B, N, C), dtype=np.float32)
    for core in range(NCORES):
        b, h = core // 2, core % 2
        y[b, h * NQ:(h + 1) * NQ, :] = res.results[core]["y"].T
    return y
